# revision 16
# baseline (speedup 1.0000x reference)
"""Trainium2 Bass kernel for 6-head causal self-attention (nn_MultiHeadAttention).

Full-input contract: kernel(**inputs) takes the unsharded numpy inputs and
returns the full [16, 2048, 384] output. Internally the batch dim (16) is
sharded 2-per-core across 8 NeuronCores (data parallel, no collectives).

Per-core pipeline (per batch):
  1. QKV projections as fp32r matmuls on x^T (pre-transposed on host).
     Q^T/K^T land head-pair-packed: partitions 0:64 = even head's d-dim,
     64:128 = odd head's, enabling K=64 row-tiled matmul pairs.
  2. Causal attention computed transposed: S^T[s, t] tiles via
     matmul(lhsT=K^T, rhs=Q^T); exp on ScalarE with fused 1/8 scale
     (scores are O(1), so softmax needs no max subtraction); causal zeroing
     via gpsimd affine_select on diagonal tiles only; U^T = V^T @ P^T via
     matmul(lhsT=[V_h | ones64], rhs=P^T) which also produces the softmax
     row-sums replicated on partitions 64:128 for free.
  3. Normalization: 1/r = exp(-ln r) on ScalarE (ln+exp share one ACT
     table set), multiply on VectorE.
  4. Output projection + bias, written transposed; host undoes transposes.
"""

import sys

for _p in ("/opt/trn_rl_repo",):
    if _p not in sys.path:
        sys.path.insert(0, _p)

import numpy as np

B, T, C = 16, 2048, 384
H, DH = 6, 64
NCORES = 8
BPC = B // NCORES  # batches per core
KC = C // 128      # 3 contraction chunks
NTQ = T // 512     # 4 query blocks
NSI = T // 128     # 16 key tiles

_CACHE = {}


def _build():
    if "nc" in _CACHE:
        return _CACHE["nc"]

    import bass_rust as _bass_rust
    import concourse.bacc as bacc
    import concourse.mybir as mybir
    import concourse.tile as tile
    from concourse.hw_specs import get_activation_tables

    dt = mybir.dt
    AF = mybir.ActivationFunctionType
    OP = mybir.AluOpType

    class _Bacc(bacc.Bacc):
        # This kernel only uses Exp and Ln on ScalarE. Both live in the
        # natural_log_exp_and_others table set; without this filter the
        # table picker alternates between exp-only and ln+exp sets,
        # inserting an ACT_TABLE_LOAD (~1.5us) per switch.
        def insert_act_table_loads(self):
            has_activation = any(
                isinstance(i, mybir.InstActivation)
                for b in self.main_func.blocks
                for i in b.instructions
            )
            if not has_activation:
                return
            keep = {"natural_log_exp_and_others"}
            tables = [
                (n, (s if n in keep else (s - {AF.Exp, AF.Ln})))
                for n, s in get_activation_tables(self.m.arch).items()
            ]
            _bass_rust.insert_act_table_loads(self, tables)

    nc = _Bacc("TRN2", target_bir_lowering=False, debug=True)

    xT_d = nc.dram_tensor("xT", [BPC, KC, 128, T], dt.bfloat16, kind="ExternalInput")
    wqk_d = nc.dram_tensor("Wqk", [KC, 128, 768], dt.bfloat16, kind="ExternalInput")
    wv_d = nc.dram_tensor("Wv", [KC, 128, 384], dt.bfloat16, kind="ExternalInput")
    wo_d = nc.dram_tensor("Wo", [KC, 128, 384], dt.bfloat16, kind="ExternalInput")
    bo_d = nc.dram_tensor("bo", [KC, 128, 1], dt.float32, kind="ExternalInput")
    yT_d = nc.dram_tensor("yT", [BPC, KC, 128, T], dt.float32, kind="ExternalOutput")

    with tile.TileContext(nc) as tc:
        with (
            tc.tile_pool(name="wp", bufs=1) as wp,
            tc.tile_pool(name="xp", bufs=2) as xp,
            tc.tile_pool(name="pp", bufs=6) as pp,
            tc.tile_pool(name="np_", bufs=4) as np_,
            tc.tile_pool(name="yp", bufs=2) as yp,
            tc.tile_pool(name="mm", bufs=4, space="PSUM") as mm,
            tc.tile_pool(name="sp", bufs=2, space="PSUM") as sp,
        ):
            # ---- constants ----
            wqk = wp.tile([128, KC, 768], dt.bfloat16, name="wqk")
            wv = wp.tile([128, KC, 384], dt.bfloat16, name="wv")
            wo = wp.tile([128, KC, 384], dt.bfloat16, name="wo")
            bo = wp.tile([128, KC], dt.float32, name="bo")
            for k in range(KC):
                nc.sync.dma_start(wqk[:, k], wqk_d[k])
                nc.sync.dma_start(wv[:, k], wv_d[k])
                nc.sync.dma_start(wo[:, k], wo_d[k])
                nc.sync.dma_start(bo[:, k, None], bo_d[k])
            # V with ones columns appended per head: [s % 128, si, h, e|ones]
            vones = wp.tile([128, NSI, H, 128], dt.bfloat16, name="vones")
            nc.gpsimd.memset(vones[:, :, :, 64:128], 1.0)

            for b in range(BPC):
                # ---- x^T load ----
                xt = xp.tile([128, KC, T], dt.bfloat16, name="xt")
                for k in range(KC):
                    nc.sync.dma_start(xt[:, k], xT_d[b, k])

                # ---- QK projections: per pair, [Q_2p|Q_2p+1] and [K_2p|K_2p+1] ----
                qt = xp.tile([128, 3, T], dt.bfloat16, name="qt")
                kt = xp.tile([128, 3, T], dt.bfloat16, name="kt")
                for p in range(3):
                    for tq in range(NTQ):
                        for qk in range(2):
                            ps = mm.tile([128, 512], dt.float32, name="ps_mm")
                            for k in range(KC):
                                nc.tensor.matmul(
                                    ps[:],
                                    wqk[:, k, 256 * p + 128 * qk : 256 * p + 128 * qk + 128],
                                    xt[:, k, 512 * tq : 512 * tq + 512],
                                    start=(k == 0),
                                    stop=(k == KC - 1),
                                )
                            dst = qt if qk == 0 else kt
                            nc.vector.tensor_copy(
                                out=dst[:, p, 512 * tq : 512 * tq + 512], in_=ps[:]
                            )

                # ---- V projection ----
                for ti in range(NSI):
                    ps = mm.tile([128, 512], dt.float32, name="ps_mm")
                    for k in range(KC):
                        nc.tensor.matmul(
                            ps[:, 0:384],
                            xt[:, k, 128 * ti : 128 * ti + 128],
                            wv[:, k, :],
                            start=(k == 0),
                            stop=(k == KC - 1),
                        )
                    nc.vector.tensor_copy(out=vones[:, ti, :, 0:64], in_=ps[:, 0:384])

                # ---- attention ----
                ot = xp.tile([128, 3, T], dt.bfloat16, name="ot")
                for p in range(3):
                    h0, h1 = 2 * p, 2 * p + 1
                    for qb in range(NTQ):
                        u0 = mm.tile([128, 512], dt.float32, name="ps_mm")
                        u1 = mm.tile([128, 512], dt.float32, name="ps_mm")
                        nsi = 4 * qb + 4
                        for si in range(nsi):
                            diag = si >= 4 * qb
                            d = si - 4 * qb if diag else 0
                            lo = 128 * d  # fully-masked columns to skip
                            sps = sp.tile([128, 1024], dt.float32, name="sps")
                            spv = sps[:].rearrange("p (h t) -> p h t", h=2)
                            for hf in range(2):
                                nc.tensor.matmul(
                                    spv[:, hf, lo:512],
                                    kt[64 * hf : 64 * hf + 64, p,
                                       128 * si : 128 * si + 128],
                                    qt[64 * hf : 64 * hf + 64, p,
                                       512 * qb + lo : 512 * qb + 512],
                                    start=True,
                                    stop=True,
                                )
                            pt = pp.tile([128, 2, 512], dt.bfloat16, name="pt")
                            if lo:
                                nc.gpsimd.memset(pt[:, :, 0:lo], 0.0)
                            nc.scalar.activation(
                                pt[:, :, lo:], spv[:, :, lo:], AF.Exp, scale=0.125
                            )
                            if diag:
                                # zero the still-masked triangle in the
                                # 128-col diagonal window: keep iff f'' >= p
                                nc.gpsimd.affine_select(
                                    out=pt[:, :, lo : lo + 128],
                                    in_=pt[:, :, lo : lo + 128],
                                    compare_op=OP.is_ge,
                                    fill=0.0,
                                    base=0,
                                    channel_multiplier=-1,
                                    pattern=[[0, 2], [1, 128]],
                                )
                            nc.tensor.matmul(
                                u0[:],
                                vones[:, si, h0, :],
                                pt[:, 0, :],
                                start=(si == 0),
                                stop=(si == nsi - 1),
                            )
                            nc.tensor.matmul(
                                u1[:],
                                vones[:, si, h1, :],
                                pt[:, 1, :],
                                start=(si == 0),
                                stop=(si == nsi - 1),
                            )
                        for hh, uu in ((h0, u0), (h1, u1)):
                            # evacuate U|r to SBUF fast so the PSUM slot frees
                            usb = np_.tile([128, 512], dt.float32, name="usb")
                            nc.vector.tensor_copy(out=usb[:], in_=uu[:])
                            lnr = np_.tile([64, 512], dt.float32, name="lnr")
                            nc.scalar.activation(lnr[:], usb[64:128, :], AF.Ln)
                            rec = np_.tile([64, 512], dt.float32, name="rec")
                            nc.scalar.activation(rec[:], lnr[:], AF.Exp, scale=-1.0)
                            nc.vector.tensor_tensor(
                                out=ot[64 * (hh % 2) : 64 * (hh % 2) + 64, p,
                                       512 * qb : 512 * qb + 512],
                                in0=usb[0:64, :],
                                in1=rec[:],
                                op=OP.mult,
                            )

                # ---- output projection + bias ----
                for tq in range(NTQ):
                    for mo in range(KC):
                        ps = mm.tile([128, 512], dt.float32, name="ps_mm")
                        for k in range(KC):
                            nc.tensor.matmul(
                                ps[:],
                                wo[:, k, 128 * mo : 128 * mo + 128],
                                ot[:, k, 512 * tq : 512 * tq + 512],
                                start=(k == 0),
                                stop=(k == KC - 1),
                            )
                        yt = yp.tile([128, 512], dt.float32, name="yt")
                        nc.vector.tensor_tensor(
                            out=yt[:],
                            in0=ps[:],
                            in1=bo[:, mo, None].to_broadcast([128, 512]),
                            op=OP.add,
                        )
                        nc.sync.dma_start(
                            yT_d[b, mo, :, 512 * tq : 512 * tq + 512], yt[:]
                        )

    nc.compile()
    _CACHE["nc"] = nc
    return nc


def _prep_inputs(x, Wq, Wk, Wv, Wo, bo):
    import ml_dtypes
    bf16 = ml_dtypes.bfloat16
    x = np.ascontiguousarray(np.asarray(x, dtype=np.float32))
    Wq = np.asarray(Wq, dtype=np.float32)
    Wk = np.asarray(Wk, dtype=np.float32)
    Wv = np.asarray(Wv, dtype=np.float32)
    Wo = np.asarray(Wo, dtype=np.float32)
    bo = np.asarray(bo, dtype=np.float32)

    # x^T: [B, T, C] -> [B, C, T] -> [B, KC, 128, T]
    xT = np.ascontiguousarray(x.transpose(0, 2, 1)).reshape(B, KC, 128, T).astype(bf16)

    # Wqk columns per pair p: [Q_2p | Q_2p+1 | K_2p | K_2p+1], 64 each
    wqk = np.empty((C, 768), np.float32)
    for p in range(3):
        wqk[:, 256 * p + 0 : 256 * p + 64] = Wq[2 * p]
        wqk[:, 256 * p + 64 : 256 * p + 128] = Wq[2 * p + 1]
        wqk[:, 256 * p + 128 : 256 * p + 192] = Wk[2 * p]
        wqk[:, 256 * p + 192 : 256 * p + 256] = Wk[2 * p + 1]
    wqk = np.ascontiguousarray(wqk.reshape(KC, 128, 768)).astype(bf16)

    # Wv columns (h*64+e), rows C -> [KC, 128, 384]
    wv = np.ascontiguousarray(
        Wv.transpose(1, 0, 2).reshape(C, H * DH).reshape(KC, 128, H * DH)
    ).astype(bf16)
    wo = np.ascontiguousarray(Wo.reshape(KC, 128, C)).astype(bf16)
    bo_r = np.ascontiguousarray(bo.reshape(KC, 128, 1))
    return xT, wqk, wv, wo, bo_r


def _run(inputs, trace=False):
    from concourse.bass_utils import run_bass_kernel_spmd

    nc = _build()
    xT, wqk, wv, wo, bo_r = _prep_inputs(**inputs)
    in_maps = [
        {
            "xT": xT[BPC * i : BPC * (i + 1)],
            "Wqk": wqk,
            "Wv": wv,
            "Wo": wo,
            "bo": bo_r,
        }
        for i in range(NCORES)
    ]
    res = run_bass_kernel_spmd(nc, in_maps, list(range(NCORES)), trace=trace)
    # yT per core: [BPC, KC, 128, T] -> full y [B, T, C]
    yT = np.concatenate([np.asarray(res.results[i]["yT"]) for i in range(NCORES)], axis=0)
    y = yT.reshape(B, C, T).transpose(0, 2, 1)
    return np.ascontiguousarray(y.astype(np.float32)), res.exec_time_ns


def kernel(**inputs) -> np.ndarray:
    y, _ = _run(inputs, trace=False)
    return y


# revision 17
# speedup vs baseline: 1.0190x; 1.0190x over previous
"""Trainium2 Bass kernel for 6-head causal self-attention (nn_MultiHeadAttention).

Full-input contract: kernel(**inputs) takes the unsharded numpy inputs and
returns the full [16, 2048, 384] output. Internally the batch dim (16) is
sharded 2-per-core across 8 NeuronCores (data parallel, no collectives).

Per-core pipeline (per batch):
  1. QKV projections as fp32r matmuls on x^T (pre-transposed on host).
     Q^T/K^T land head-pair-packed: partitions 0:64 = even head's d-dim,
     64:128 = odd head's, enabling K=64 row-tiled matmul pairs.
  2. Causal attention computed transposed: S^T[s, t] tiles via
     matmul(lhsT=K^T, rhs=Q^T); exp on ScalarE with fused 1/8 scale
     (scores are O(1), so softmax needs no max subtraction); causal zeroing
     via gpsimd affine_select on diagonal tiles only; U^T = V^T @ P^T via
     matmul(lhsT=[V_h | ones64], rhs=P^T) which also produces the softmax
     row-sums replicated on partitions 64:128 for free.
  3. Normalization: 1/r = exp(-ln r) on ScalarE (ln+exp share one ACT
     table set), multiply on VectorE.
  4. Output projection + bias, written transposed; host undoes transposes.
"""

import sys

for _p in ("/opt/trn_rl_repo",):
    if _p not in sys.path:
        sys.path.insert(0, _p)

import numpy as np

B, T, C = 16, 2048, 384
H, DH = 6, 64
NCORES = 8
BPC = B // NCORES  # batches per core
KC = C // 128      # 3 contraction chunks
NTQ = T // 512     # 4 query blocks
NSI = T // 128     # 16 key tiles

_CACHE = {}


def _build():
    if "nc" in _CACHE:
        return _CACHE["nc"]

    import bass_rust as _bass_rust
    import concourse.bacc as bacc
    import concourse.mybir as mybir
    import concourse.tile as tile
    from concourse.hw_specs import get_activation_tables

    dt = mybir.dt
    AF = mybir.ActivationFunctionType
    OP = mybir.AluOpType

    class _Bacc(bacc.Bacc):
        # This kernel only uses Exp and Ln on ScalarE. Both live in the
        # natural_log_exp_and_others table set; without this filter the
        # table picker alternates between exp-only and ln+exp sets,
        # inserting an ACT_TABLE_LOAD (~1.5us) per switch.
        def insert_act_table_loads(self):
            has_activation = any(
                isinstance(i, mybir.InstActivation)
                for b in self.main_func.blocks
                for i in b.instructions
            )
            if not has_activation:
                return
            keep = {"natural_log_exp_and_others"}
            tables = [
                (n, (s if n in keep else (s - {AF.Exp, AF.Ln})))
                for n, s in get_activation_tables(self.m.arch).items()
            ]
            _bass_rust.insert_act_table_loads(self, tables)

    nc = _Bacc("TRN2", target_bir_lowering=False, debug=True)

    xT_d = nc.dram_tensor("xT", [BPC, KC, 128, T], dt.bfloat16, kind="ExternalInput")
    wqk_d = nc.dram_tensor("Wqk", [KC, 128, 768], dt.bfloat16, kind="ExternalInput")
    wv_d = nc.dram_tensor("Wv", [KC, 128, 384], dt.bfloat16, kind="ExternalInput")
    wo_d = nc.dram_tensor("Wo", [KC, 128, 384], dt.bfloat16, kind="ExternalInput")
    bo_d = nc.dram_tensor("bo", [KC, 128, 1], dt.float32, kind="ExternalInput")
    yT_d = nc.dram_tensor("yT", [BPC, KC, 128, T], dt.float32, kind="ExternalOutput")

    with tile.TileContext(nc) as tc:
        with (
            tc.tile_pool(name="wp", bufs=1) as wp,
            tc.tile_pool(name="xp", bufs=2) as xp,
            tc.tile_pool(name="pp", bufs=6) as pp,
            tc.tile_pool(name="np_", bufs=4) as np_,
            tc.tile_pool(name="yp", bufs=2) as yp,
            tc.tile_pool(name="mm", bufs=2, space="PSUM") as mm,
            tc.tile_pool(name="sp", bufs=3, space="PSUM") as sp,
        ):
            # ---- constants ----
            wqk = wp.tile([128, KC, 768], dt.bfloat16, name="wqk")
            wv = wp.tile([128, KC, 384], dt.bfloat16, name="wv")
            wo = wp.tile([128, KC, 384], dt.bfloat16, name="wo")
            bo = wp.tile([128, KC], dt.float32, name="bo")
            for k in range(KC):
                nc.sync.dma_start(wqk[:, k], wqk_d[k])
                nc.sync.dma_start(wv[:, k], wv_d[k])
                nc.sync.dma_start(wo[:, k], wo_d[k])
                nc.sync.dma_start(bo[:, k, None], bo_d[k])
            # V with ones columns appended per head: [s % 128, si, h, e|ones]
            vones = wp.tile([128, NSI, H, 128], dt.bfloat16, name="vones")
            nc.gpsimd.memset(vones[:, :, :, 64:128], 1.0)

            for b in range(BPC):
                # ---- x^T load ----
                xt = xp.tile([128, KC, T], dt.bfloat16, name="xt")
                for k in range(KC):
                    nc.sync.dma_start(xt[:, k], xT_d[b, k])

                # ---- QK projections: per pair, [Q_2p|Q_2p+1] and [K_2p|K_2p+1] ----
                qt = xp.tile([128, 3, T], dt.bfloat16, name="qt")
                kt = xp.tile([128, 3, T], dt.bfloat16, name="kt")
                for p in range(3):
                    for tq in range(NTQ):
                        for qk in range(2):
                            ps = mm.tile([128, 512], dt.float32, name="ps_mm")
                            for k in range(KC):
                                nc.tensor.matmul(
                                    ps[:],
                                    wqk[:, k, 256 * p + 128 * qk : 256 * p + 128 * qk + 128],
                                    xt[:, k, 512 * tq : 512 * tq + 512],
                                    start=(k == 0),
                                    stop=(k == KC - 1),
                                )
                            dst = qt if qk == 0 else kt
                            nc.vector.tensor_copy(
                                out=dst[:, p, 512 * tq : 512 * tq + 512], in_=ps[:]
                            )

                # ---- V projection ----
                for ti in range(NSI):
                    ps = mm.tile([128, 512], dt.float32, name="ps_mm")
                    for k in range(KC):
                        nc.tensor.matmul(
                            ps[:, 0:384],
                            xt[:, k, 128 * ti : 128 * ti + 128],
                            wv[:, k, :],
                            start=(k == 0),
                            stop=(k == KC - 1),
                        )
                    nc.vector.tensor_copy(out=vones[:, ti, :, 0:64], in_=ps[:, 0:384])

                # ---- attention ----
                ot = xp.tile([128, 3, T], dt.bfloat16, name="ot")
                for p in range(3):
                    h0, h1 = 2 * p, 2 * p + 1
                    for qb in range(NTQ):
                        u0 = mm.tile([128, 512], dt.float32, name="ps_mm")
                        u1 = mm.tile([128, 512], dt.float32, name="ps_mm")
                        nsi = 4 * qb + 4
                        for si in range(nsi):
                            diag = si >= 4 * qb
                            d = si - 4 * qb if diag else 0
                            lo = 128 * d  # fully-masked columns to skip
                            sps = sp.tile([128, 1024], dt.float32, name="sps")
                            spv = sps[:].rearrange("p (h t) -> p h t", h=2)
                            for hf in range(2):
                                nc.tensor.matmul(
                                    spv[:, hf, lo:512],
                                    kt[64 * hf : 64 * hf + 64, p,
                                       128 * si : 128 * si + 128],
                                    qt[64 * hf : 64 * hf + 64, p,
                                       512 * qb + lo : 512 * qb + 512],
                                    start=True,
                                    stop=True,
                                )
                            pt = pp.tile([128, 2, 512], dt.bfloat16, name="pt")
                            if lo:
                                nc.gpsimd.memset(pt[:, :, 0:lo], 0.0)
                            nc.scalar.activation(
                                pt[:, :, lo:], spv[:, :, lo:], AF.Exp, scale=0.125
                            )
                            if diag:
                                # zero the still-masked triangle in the
                                # 128-col diagonal window: keep iff f'' >= p
                                nc.gpsimd.affine_select(
                                    out=pt[:, :, lo : lo + 128],
                                    in_=pt[:, :, lo : lo + 128],
                                    compare_op=OP.is_ge,
                                    fill=0.0,
                                    base=0,
                                    channel_multiplier=-1,
                                    pattern=[[0, 2], [1, 128]],
                                )
                            nc.tensor.matmul(
                                u0[:],
                                vones[:, si, h0, :],
                                pt[:, 0, :],
                                start=(si == 0),
                                stop=(si == nsi - 1),
                            )
                            nc.tensor.matmul(
                                u1[:],
                                vones[:, si, h1, :],
                                pt[:, 1, :],
                                start=(si == 0),
                                stop=(si == nsi - 1),
                            )
                        for hh, uu in ((h0, u0), (h1, u1)):
                            # evacuate U|r to SBUF fast so the PSUM slot frees
                            usb = np_.tile([128, 512], dt.float32, name="usb")
                            nc.vector.tensor_copy(out=usb[:], in_=uu[:])
                            lnr = np_.tile([64, 512], dt.float32, name="lnr")
                            nc.scalar.activation(lnr[:], usb[64:128, :], AF.Ln)
                            rec = np_.tile([64, 512], dt.float32, name="rec")
                            nc.scalar.activation(rec[:], lnr[:], AF.Exp, scale=-1.0)
                            nc.vector.tensor_tensor(
                                out=ot[64 * (hh % 2) : 64 * (hh % 2) + 64, p,
                                       512 * qb : 512 * qb + 512],
                                in0=usb[0:64, :],
                                in1=rec[:],
                                op=OP.mult,
                            )

                # ---- output projection + bias ----
                for tq in range(NTQ):
                    for mo in range(KC):
                        ps = mm.tile([128, 512], dt.float32, name="ps_mm")
                        for k in range(KC):
                            nc.tensor.matmul(
                                ps[:],
                                wo[:, k, 128 * mo : 128 * mo + 128],
                                ot[:, k, 512 * tq : 512 * tq + 512],
                                start=(k == 0),
                                stop=(k == KC - 1),
                            )
                        yt = yp.tile([128, 512], dt.float32, name="yt")
                        nc.vector.tensor_tensor(
                            out=yt[:],
                            in0=ps[:],
                            in1=bo[:, mo, None].to_broadcast([128, 512]),
                            op=OP.add,
                        )
                        nc.sync.dma_start(
                            yT_d[b, mo, :, 512 * tq : 512 * tq + 512], yt[:]
                        )

    nc.compile()
    _CACHE["nc"] = nc
    return nc


def _prep_inputs(x, Wq, Wk, Wv, Wo, bo):
    import ml_dtypes
    bf16 = ml_dtypes.bfloat16
    x = np.ascontiguousarray(np.asarray(x, dtype=np.float32))
    Wq = np.asarray(Wq, dtype=np.float32)
    Wk = np.asarray(Wk, dtype=np.float32)
    Wv = np.asarray(Wv, dtype=np.float32)
    Wo = np.asarray(Wo, dtype=np.float32)
    bo = np.asarray(bo, dtype=np.float32)

    # x^T: [B, T, C] -> [B, C, T] -> [B, KC, 128, T]
    xT = np.ascontiguousarray(x.transpose(0, 2, 1)).reshape(B, KC, 128, T).astype(bf16)

    # Wqk columns per pair p: [Q_2p | Q_2p+1 | K_2p | K_2p+1], 64 each
    wqk = np.empty((C, 768), np.float32)
    for p in range(3):
        wqk[:, 256 * p + 0 : 256 * p + 64] = Wq[2 * p]
        wqk[:, 256 * p + 64 : 256 * p + 128] = Wq[2 * p + 1]
        wqk[:, 256 * p + 128 : 256 * p + 192] = Wk[2 * p]
        wqk[:, 256 * p + 192 : 256 * p + 256] = Wk[2 * p + 1]
    wqk = np.ascontiguousarray(wqk.reshape(KC, 128, 768)).astype(bf16)

    # Wv columns (h*64+e), rows C -> [KC, 128, 384]
    wv = np.ascontiguousarray(
        Wv.transpose(1, 0, 2).reshape(C, H * DH).reshape(KC, 128, H * DH)
    ).astype(bf16)
    wo = np.ascontiguousarray(Wo.reshape(KC, 128, C)).astype(bf16)
    bo_r = np.ascontiguousarray(bo.reshape(KC, 128, 1))
    return xT, wqk, wv, wo, bo_r


def _run(inputs, trace=False):
    from concourse.bass_utils import run_bass_kernel_spmd

    nc = _build()
    xT, wqk, wv, wo, bo_r = _prep_inputs(**inputs)
    in_maps = [
        {
            "xT": xT[BPC * i : BPC * (i + 1)],
            "Wqk": wqk,
            "Wv": wv,
            "Wo": wo,
            "bo": bo_r,
        }
        for i in range(NCORES)
    ]
    res = run_bass_kernel_spmd(nc, in_maps, list(range(NCORES)), trace=trace)
    # yT per core: [BPC, KC, 128, T] -> full y [B, T, C]
    yT = np.concatenate([np.asarray(res.results[i]["yT"]) for i in range(NCORES)], axis=0)
    y = yT.reshape(B, C, T).transpose(0, 2, 1)
    return np.ascontiguousarray(y.astype(np.float32)), res.exec_time_ns


def kernel(**inputs) -> np.ndarray:
    y, _ = _run(inputs, trace=False)
    return y


# revision 19
# speedup vs baseline: 1.0195x; 1.0005x over previous
"""Trainium2 Bass kernel for 6-head causal self-attention (nn_MultiHeadAttention).

Full-input contract: kernel(**inputs) takes the unsharded numpy inputs and
returns the full [16, 2048, 384] output. Internally the batch dim (16) is
sharded 2-per-core across 8 NeuronCores (data parallel, no collectives).

Per-core pipeline (per batch):
  1. QKV projections as fp32r matmuls on x^T (pre-transposed on host).
     Q^T/K^T land head-pair-packed: partitions 0:64 = even head's d-dim,
     64:128 = odd head's, enabling K=64 row-tiled matmul pairs.
  2. Causal attention computed transposed: S^T[s, t] tiles via
     matmul(lhsT=K^T, rhs=Q^T); exp on ScalarE with fused 1/8 scale
     (scores are O(1), so softmax needs no max subtraction); causal zeroing
     via gpsimd affine_select on diagonal tiles only; U^T = V^T @ P^T via
     matmul(lhsT=[V_h | ones64], rhs=P^T) which also produces the softmax
     row-sums replicated on partitions 64:128 for free.
  3. Normalization: 1/r = exp(-ln r) on ScalarE (ln+exp share one ACT
     table set), multiply on VectorE.
  4. Output projection + bias, written transposed; host undoes transposes.
"""

import sys

for _p in ("/opt/trn_rl_repo",):
    if _p not in sys.path:
        sys.path.insert(0, _p)

import numpy as np

B, T, C = 16, 2048, 384
H, DH = 6, 64
NCORES = 8
BPC = B // NCORES  # batches per core
KC = C // 128      # 3 contraction chunks
NTQ = T // 512     # 4 query blocks
NSI = T // 128     # 16 key tiles

_CACHE = {}


def _build():
    if "nc" in _CACHE:
        return _CACHE["nc"]

    import bass_rust as _bass_rust
    import concourse.bacc as bacc
    import concourse.mybir as mybir
    import concourse.tile as tile
    from concourse.hw_specs import get_activation_tables

    dt = mybir.dt
    AF = mybir.ActivationFunctionType
    OP = mybir.AluOpType

    class _Bacc(bacc.Bacc):
        # This kernel only uses Exp and Ln on ScalarE. Both live in the
        # natural_log_exp_and_others table set; without this filter the
        # table picker alternates between exp-only and ln+exp sets,
        # inserting an ACT_TABLE_LOAD (~1.5us) per switch.
        def insert_act_table_loads(self):
            has_activation = any(
                isinstance(i, mybir.InstActivation)
                for b in self.main_func.blocks
                for i in b.instructions
            )
            if not has_activation:
                return
            keep = {"natural_log_exp_and_others"}
            tables = [
                (n, (s if n in keep else (s - {AF.Exp, AF.Ln})))
                for n, s in get_activation_tables(self.m.arch).items()
            ]
            _bass_rust.insert_act_table_loads(self, tables)

    nc = _Bacc("TRN2", target_bir_lowering=False, debug=True)

    xT_d = nc.dram_tensor("xT", [BPC, KC, 128, T], dt.bfloat16, kind="ExternalInput")
    wqk_d = nc.dram_tensor("Wqk", [KC, 128, 768], dt.bfloat16, kind="ExternalInput")
    wv_d = nc.dram_tensor("Wv", [KC, 128, 384], dt.bfloat16, kind="ExternalInput")
    wo_d = nc.dram_tensor("Wo", [KC, 128, 384], dt.bfloat16, kind="ExternalInput")
    bo_d = nc.dram_tensor("bo", [KC, 128, 1], dt.float32, kind="ExternalInput")
    yT_d = nc.dram_tensor("yT", [BPC, KC, 128, T], dt.float32, kind="ExternalOutput")

    with tile.TileContext(nc) as tc:
        with (
            tc.tile_pool(name="wp", bufs=1) as wp,
            tc.tile_pool(name="xp", bufs=2) as xp,
            tc.tile_pool(name="pp", bufs=6) as pp,
            tc.tile_pool(name="np_", bufs=4) as np_,
            tc.tile_pool(name="yp", bufs=2) as yp,
            tc.tile_pool(name="mm", bufs=2, space="PSUM") as mm,
            tc.tile_pool(name="sp", bufs=3, space="PSUM") as sp,
        ):
            # ---- constants ----
            wqk = wp.tile([128, KC, 768], dt.bfloat16, name="wqk")
            wv = wp.tile([128, KC, 384], dt.bfloat16, name="wv")
            wo = wp.tile([128, KC, 384], dt.bfloat16, name="wo")
            bo = wp.tile([128, KC], dt.float32, name="bo")
            for k in range(KC):
                nc.sync.dma_start(wqk[:, k], wqk_d[k])
                nc.sync.dma_start(wv[:, k], wv_d[k])
                nc.sync.dma_start(wo[:, k], wo_d[k])
                nc.sync.dma_start(bo[:, k, None], bo_d[k])
            # V with ones columns appended per head: [s % 128, si, h, e|ones]
            vones = wp.tile([128, NSI, H, 128], dt.bfloat16, name="vones")
            nc.gpsimd.memset(vones[:, :, :, 64:128], 1.0)

            for b in range(BPC):
                # ---- x^T load ----
                xt = xp.tile([128, KC, T], dt.bfloat16, name="xt")
                for k in range(KC):
                    nc.sync.dma_start(xt[:, k], xT_d[b, k])

                # ---- QK projections: per pair, [Q_2p|Q_2p+1] and [K_2p|K_2p+1] ----
                qt = xp.tile([128, 3, T], dt.bfloat16, name="qt")
                kt = xp.tile([128, 3, T], dt.bfloat16, name="kt")
                for p in range(3):
                    for tq in range(NTQ):
                        for qk in range(2):
                            ps = mm.tile([128, 512], dt.float32, name="ps_mm")
                            for k in range(KC):
                                nc.tensor.matmul(
                                    ps[:],
                                    wqk[:, k, 256 * p + 128 * qk : 256 * p + 128 * qk + 128],
                                    xt[:, k, 512 * tq : 512 * tq + 512],
                                    start=(k == 0),
                                    stop=(k == KC - 1),
                                )
                            dst = qt if qk == 0 else kt
                            nc.vector.tensor_copy(
                                out=dst[:, p, 512 * tq : 512 * tq + 512], in_=ps[:]
                            )

                # ---- V projection ----
                for ti in range(NSI):
                    ps = mm.tile([128, 512], dt.float32, name="ps_mm")
                    for k in range(KC):
                        nc.tensor.matmul(
                            ps[:, 0:384],
                            xt[:, k, 128 * ti : 128 * ti + 128],
                            wv[:, k, :],
                            start=(k == 0),
                            stop=(k == KC - 1),
                        )
                    nc.vector.tensor_copy(out=vones[:, ti, :, 0:64], in_=ps[:, 0:384])

                # ---- attention ----
                ot = xp.tile([128, 3, T], dt.bfloat16, name="ot")
                for p in range(3):
                    h0, h1 = 2 * p, 2 * p + 1
                    for qb in range(NTQ):
                        u0 = mm.tile([128, 512], dt.float32, name="ps_mm")
                        u1 = mm.tile([128, 512], dt.float32, name="ps_mm")
                        nsi = 4 * qb + 4
                        pts = {}

                        def emit_u(si, nsi=nsi, u0=u0, u1=u1, pts=pts):
                            pt = pts.pop(si)
                            for hf, uu in ((0, u0), (1, u1)):
                                nc.tensor.matmul(
                                    uu[:],
                                    vones[:, si, 2 * p + hf, :],
                                    pt[:, hf, :],
                                    start=(si == 0),
                                    stop=(si == nsi - 1),
                                )

                        for si in range(nsi):
                            diag = si >= 4 * qb
                            d = si - 4 * qb if diag else 0
                            lo = 128 * d  # fully-masked columns to skip
                            sps = sp.tile([128, 1024], dt.float32, name="sps")
                            spv = sps[:].rearrange("p (h t) -> p h t", h=2)
                            for hf in range(2):
                                nc.tensor.matmul(
                                    spv[:, hf, lo:512],
                                    kt[64 * hf : 64 * hf + 64, p,
                                       128 * si : 128 * si + 128],
                                    qt[64 * hf : 64 * hf + 64, p,
                                       512 * qb + lo : 512 * qb + 512],
                                    start=True,
                                    stop=True,
                                )
                            pt = pp.tile([128, 2, 512], dt.bfloat16, name="pt")
                            if lo:
                                nc.gpsimd.memset(pt[:, :, 0:lo], 0.0)
                            nc.scalar.activation(
                                pt[:, :, lo:], spv[:, :, lo:], AF.Exp, scale=0.125
                            )
                            if diag:
                                # zero the still-masked triangle in the
                                # 128-col diagonal window: keep iff f'' >= p
                                nc.gpsimd.affine_select(
                                    out=pt[:, :, lo : lo + 128],
                                    in_=pt[:, :, lo : lo + 128],
                                    compare_op=OP.is_ge,
                                    fill=0.0,
                                    base=0,
                                    channel_multiplier=-1,
                                    pattern=[[0, 2], [1, 128]],
                                )
                            pts[si] = pt
                            # software pipeline: U-matmuls run one si behind
                            # the S-matmuls so exp(si) overlaps S(si+1) on PE
                            if si > 0:
                                emit_u(si - 1)
                        emit_u(nsi - 1)
                        for hh, uu in ((h0, u0), (h1, u1)):
                            # evacuate U|r to SBUF fast so the PSUM slot frees
                            usb = np_.tile([128, 512], dt.float32, name="usb")
                            nc.vector.tensor_copy(out=usb[:], in_=uu[:])
                            lnr = np_.tile([64, 512], dt.float32, name="lnr")
                            nc.scalar.activation(lnr[:], usb[64:128, :], AF.Ln)
                            rec = np_.tile([64, 512], dt.float32, name="rec")
                            nc.scalar.activation(rec[:], lnr[:], AF.Exp, scale=-1.0)
                            nc.vector.tensor_tensor(
                                out=ot[64 * (hh % 2) : 64 * (hh % 2) + 64, p,
                                       512 * qb : 512 * qb + 512],
                                in0=usb[0:64, :],
                                in1=rec[:],
                                op=OP.mult,
                            )

                # ---- output projection + bias ----
                for tq in range(NTQ):
                    for mo in range(KC):
                        ps = mm.tile([128, 512], dt.float32, name="ps_mm")
                        for k in range(KC):
                            nc.tensor.matmul(
                                ps[:],
                                wo[:, k, 128 * mo : 128 * mo + 128],
                                ot[:, k, 512 * tq : 512 * tq + 512],
                                start=(k == 0),
                                stop=(k == KC - 1),
                            )
                        yt = yp.tile([128, 512], dt.float32, name="yt")
                        nc.vector.tensor_tensor(
                            out=yt[:],
                            in0=ps[:],
                            in1=bo[:, mo, None].to_broadcast([128, 512]),
                            op=OP.add,
                        )
                        nc.sync.dma_start(
                            yT_d[b, mo, :, 512 * tq : 512 * tq + 512], yt[:]
                        )

    nc.compile()
    _CACHE["nc"] = nc
    return nc


def _prep_inputs(x, Wq, Wk, Wv, Wo, bo):
    import ml_dtypes
    bf16 = ml_dtypes.bfloat16
    x = np.ascontiguousarray(np.asarray(x, dtype=np.float32))
    Wq = np.asarray(Wq, dtype=np.float32)
    Wk = np.asarray(Wk, dtype=np.float32)
    Wv = np.asarray(Wv, dtype=np.float32)
    Wo = np.asarray(Wo, dtype=np.float32)
    bo = np.asarray(bo, dtype=np.float32)

    # x^T: [B, T, C] -> [B, C, T] -> [B, KC, 128, T]
    xT = np.ascontiguousarray(x.transpose(0, 2, 1)).reshape(B, KC, 128, T).astype(bf16)

    # Wqk columns per pair p: [Q_2p | Q_2p+1 | K_2p | K_2p+1], 64 each
    wqk = np.empty((C, 768), np.float32)
    for p in range(3):
        wqk[:, 256 * p + 0 : 256 * p + 64] = Wq[2 * p]
        wqk[:, 256 * p + 64 : 256 * p + 128] = Wq[2 * p + 1]
        wqk[:, 256 * p + 128 : 256 * p + 192] = Wk[2 * p]
        wqk[:, 256 * p + 192 : 256 * p + 256] = Wk[2 * p + 1]
    wqk = np.ascontiguousarray(wqk.reshape(KC, 128, 768)).astype(bf16)

    # Wv columns (h*64+e), rows C -> [KC, 128, 384]
    wv = np.ascontiguousarray(
        Wv.transpose(1, 0, 2).reshape(C, H * DH).reshape(KC, 128, H * DH)
    ).astype(bf16)
    wo = np.ascontiguousarray(Wo.reshape(KC, 128, C)).astype(bf16)
    bo_r = np.ascontiguousarray(bo.reshape(KC, 128, 1))
    return xT, wqk, wv, wo, bo_r


def _run(inputs, trace=False):
    from concourse.bass_utils import run_bass_kernel_spmd

    nc = _build()
    xT, wqk, wv, wo, bo_r = _prep_inputs(**inputs)
    in_maps = [
        {
            "xT": xT[BPC * i : BPC * (i + 1)],
            "Wqk": wqk,
            "Wv": wv,
            "Wo": wo,
            "bo": bo_r,
        }
        for i in range(NCORES)
    ]
    res = run_bass_kernel_spmd(nc, in_maps, list(range(NCORES)), trace=trace)
    # yT per core: [BPC, KC, 128, T] -> full y [B, T, C]
    yT = np.concatenate([np.asarray(res.results[i]["yT"]) for i in range(NCORES)], axis=0)
    y = yT.reshape(B, C, T).transpose(0, 2, 1)
    return np.ascontiguousarray(y.astype(np.float32)), res.exec_time_ns


def kernel(**inputs) -> np.ndarray:
    y, _ = _run(inputs, trace=False)
    return y


# revision 20
# speedup vs baseline: 1.0541x; 1.0340x over previous
"""Trainium2 Bass kernel for 6-head causal self-attention (nn_MultiHeadAttention).

Full-input contract: kernel(**inputs) takes the unsharded numpy inputs and
returns the full [16, 2048, 384] output. Internally the batch dim (16) is
sharded 2-per-core across 8 NeuronCores (data parallel, no collectives).

Per-core pipeline (per batch):
  1. QKV projections as fp32r matmuls on x^T (pre-transposed on host).
     Q^T/K^T land head-pair-packed: partitions 0:64 = even head's d-dim,
     64:128 = odd head's, enabling K=64 row-tiled matmul pairs.
  2. Causal attention computed transposed: S^T[s, t] tiles via
     matmul(lhsT=K^T, rhs=Q^T); exp on ScalarE with fused 1/8 scale
     (scores are O(1), so softmax needs no max subtraction); causal zeroing
     via gpsimd affine_select on diagonal tiles only; U^T = V^T @ P^T via
     matmul(lhsT=[V_h | ones64], rhs=P^T) which also produces the softmax
     row-sums replicated on partitions 64:128 for free.
  3. Normalization: 1/r = exp(-ln r) on ScalarE (ln+exp share one ACT
     table set), multiply on VectorE.
  4. Output projection + bias, written transposed; host undoes transposes.
"""

import sys

for _p in ("/opt/trn_rl_repo",):
    if _p not in sys.path:
        sys.path.insert(0, _p)

import numpy as np

B, T, C = 16, 2048, 384
H, DH = 6, 64
NCORES = 8
BPC = B // NCORES  # batches per core
KC = C // 128      # 3 contraction chunks
NTQ = T // 512     # 4 query blocks
NSI = T // 128     # 16 key tiles

_CACHE = {}


def _build():
    if "nc" in _CACHE:
        return _CACHE["nc"]

    import bass_rust as _bass_rust
    import concourse.bacc as bacc
    import concourse.mybir as mybir
    import concourse.tile as tile
    from concourse.hw_specs import get_activation_tables

    dt = mybir.dt
    AF = mybir.ActivationFunctionType
    OP = mybir.AluOpType

    class _Bacc(bacc.Bacc):
        # This kernel only uses Exp and Ln on ScalarE. Both live in the
        # natural_log_exp_and_others table set; without this filter the
        # table picker alternates between exp-only and ln+exp sets,
        # inserting an ACT_TABLE_LOAD (~1.5us) per switch.
        def insert_act_table_loads(self):
            has_activation = any(
                isinstance(i, mybir.InstActivation)
                for b in self.main_func.blocks
                for i in b.instructions
            )
            if not has_activation:
                return
            keep = {"natural_log_exp_and_others"}
            tables = [
                (n, (s if n in keep else (s - {AF.Exp, AF.Ln})))
                for n, s in get_activation_tables(self.m.arch).items()
            ]
            _bass_rust.insert_act_table_loads(self, tables)

    nc = _Bacc("TRN2", target_bir_lowering=False, debug=True)

    xT_d = nc.dram_tensor("xT", [BPC, KC, 128, T], dt.bfloat16, kind="ExternalInput")
    wqk_d = nc.dram_tensor("Wqk", [KC, 128, 768], dt.bfloat16, kind="ExternalInput")
    wv_d = nc.dram_tensor("Wv", [KC, 128, 384], dt.bfloat16, kind="ExternalInput")
    wo_d = nc.dram_tensor("Wo", [KC, 128, 384], dt.bfloat16, kind="ExternalInput")
    bo_d = nc.dram_tensor("bo", [KC, 128, 1], dt.float32, kind="ExternalInput")
    yT_d = nc.dram_tensor("yT", [BPC, KC, 128, T], dt.float32, kind="ExternalOutput")

    with tile.TileContext(nc) as tc:
        with (
            tc.tile_pool(name="wp", bufs=1) as wp,
            tc.tile_pool(name="vp", bufs=2) as vp,
            tc.tile_pool(name="xp", bufs=2) as xp,
            tc.tile_pool(name="pp", bufs=6) as pp,
            tc.tile_pool(name="np_", bufs=3) as np_,
            tc.tile_pool(name="yp", bufs=2) as yp,
            tc.tile_pool(name="ups", bufs=2, space="PSUM") as ups,
            tc.tile_pool(name="mm", bufs=2, space="PSUM") as mm,
            tc.tile_pool(name="sp", bufs=2, space="PSUM") as sp,
        ):
            # ---- constants ----
            wqk = wp.tile([128, KC, 768], dt.bfloat16, name="wqk")
            wv = wp.tile([128, KC, 384], dt.bfloat16, name="wv")
            wo = wp.tile([128, KC, 384], dt.bfloat16, name="wo")
            bo = wp.tile([128, KC], dt.float32, name="bo")
            for k in range(KC):
                nc.sync.dma_start(wqk[:, k], wqk_d[k])
                nc.sync.dma_start(wv[:, k], wv_d[k])
                nc.sync.dma_start(wo[:, k], wo_d[k])
                nc.sync.dma_start(bo[:, k, None], bo_d[k])

            # Deferred PE work queue: projection matmuls are drained one
            # unit per attention si-step so the (ACT-bound) attention loop
            # hides the (PE-only) projection phases.
            fillers = []

            def drain(n=1):
                for _ in range(n):
                    if fillers:
                        fillers.pop(0)()

            def flush():
                while fillers:
                    fillers.pop(0)()

            def load_x(b):
                xt = xp.tile([128, KC, T], dt.bfloat16, name="xt")
                for k in range(KC):
                    nc.sync.dma_start(xt[:, k], xT_d[b, k])
                return xt

            def new_vones():
                # V with ones columns per head: [s % 128, si, h, e|ones]
                vones = vp.tile([128, NSI, H, 128], dt.bfloat16, name="vones")
                nc.gpsimd.memset(vones[:, :, :, 64:128], 1.0)
                return vones

            def v_unit(xt, vones, ti):
                def emit():
                    ps = mm.tile([128, 512], dt.float32, name="ps_mm")
                    for k in range(KC):
                        nc.tensor.matmul(
                            ps[:, 0:384],
                            xt[:, k, 128 * ti : 128 * ti + 128],
                            wv[:, k, :],
                            start=(k == 0),
                            stop=(k == KC - 1),
                        )
                    nc.vector.tensor_copy(
                        out=vones[:, ti, :, 0:64], in_=ps[:, 0:384]
                    )
                return emit

            def qk_unit(xt, qt, kt, p, tq, qk):
                def emit():
                    ps = mm.tile([128, 512], dt.float32, name="ps_mm")
                    for k in range(KC):
                        nc.tensor.matmul(
                            ps[:],
                            wqk[:, k, 256 * p + 128 * qk : 256 * p + 128 * qk + 128],
                            xt[:, k, 512 * tq : 512 * tq + 512],
                            start=(k == 0),
                            stop=(k == KC - 1),
                        )
                    dst = qt if qk == 0 else kt
                    nc.vector.tensor_copy(
                        out=dst[:, p, 512 * tq : 512 * tq + 512], in_=ps[:]
                    )
                return emit

            def oproj_unit(b, ot, tq, mo):
                def emit():
                    ps = mm.tile([128, 512], dt.float32, name="ps_mm")
                    for k in range(KC):
                        nc.tensor.matmul(
                            ps[:],
                            wo[:, k, 128 * mo : 128 * mo + 128],
                            ot[:, k, 512 * tq : 512 * tq + 512],
                            start=(k == 0),
                            stop=(k == KC - 1),
                        )
                    yt = yp.tile([128, 512], dt.float32, name="yt")
                    nc.vector.tensor_tensor(
                        out=yt[:],
                        in0=ps[:],
                        in1=bo[:, mo, None].to_broadcast([128, 512]),
                        op=OP.add,
                    )
                    nc.sync.dma_start(
                        yT_d[b, mo, :, 512 * tq : 512 * tq + 512], yt[:]
                    )
                return emit

            def attention_pair(qt, kt, vones, ot, p):
                for qb in range(NTQ):
                    u0 = ups.tile([128, 512], dt.float32, name="ps_u")
                    u1 = ups.tile([128, 512], dt.float32, name="ps_u")
                    nsi = 4 * qb + 4
                    pts = {}

                    def emit_u(si, nsi=nsi, u0=u0, u1=u1, pts=pts):
                        pt = pts.pop(si)
                        for hf, uu in ((0, u0), (1, u1)):
                            nc.tensor.matmul(
                                uu[:],
                                vones[:, si, 2 * p + hf, :],
                                pt[:, hf, :],
                                start=(si == 0),
                                stop=(si == nsi - 1),
                            )

                    for si in range(nsi):
                        diag = si >= 4 * qb
                        d = si - 4 * qb if diag else 0
                        lo = 128 * d  # fully-masked columns to skip
                        sps = sp.tile([128, 1024], dt.float32, name="sps")
                        spv = sps[:].rearrange("p (h t) -> p h t", h=2)
                        for hf in range(2):
                            nc.tensor.matmul(
                                spv[:, hf, lo:512],
                                kt[64 * hf : 64 * hf + 64, p,
                                   128 * si : 128 * si + 128],
                                qt[64 * hf : 64 * hf + 64, p,
                                   512 * qb + lo : 512 * qb + 512],
                                start=True,
                                stop=True,
                            )
                        pt = pp.tile([128, 2, 512], dt.bfloat16, name="pt")
                        if lo:
                            nc.gpsimd.memset(pt[:, :, 0:lo], 0.0)
                        nc.scalar.activation(
                            pt[:, :, lo:], spv[:, :, lo:], AF.Exp, scale=0.125
                        )
                        if diag:
                            # zero the still-masked triangle in the 128-col
                            # diagonal window: keep iff f >= p
                            nc.gpsimd.affine_select(
                                out=pt[:, :, lo : lo + 128],
                                in_=pt[:, :, lo : lo + 128],
                                compare_op=OP.is_ge,
                                fill=0.0,
                                base=0,
                                channel_multiplier=-1,
                                pattern=[[0, 2], [1, 128]],
                            )
                        pts[si] = pt
                        # software pipeline: U-matmuls one si behind the
                        # S-matmuls so exp(si) overlaps S(si+1) on PE
                        if si > 0:
                            emit_u(si - 1)
                        drain(1)
                    emit_u(nsi - 1)
                    for hh, uu in ((2 * p, u0), (2 * p + 1, u1)):
                        # evacuate U|r to SBUF fast so the PSUM slot frees
                        usb = np_.tile([128, 512], dt.float32, name="usb")
                        nc.vector.tensor_copy(out=usb[:], in_=uu[:])
                        lnr = np_.tile([64, 512], dt.float32, name="lnr")
                        nc.scalar.activation(lnr[:], usb[64:128, :], AF.Ln)
                        rec = np_.tile([64, 512], dt.float32, name="rec")
                        nc.scalar.activation(rec[:], lnr[:], AF.Exp, scale=-1.0)
                        nc.vector.tensor_tensor(
                            out=ot[64 * (hh % 2) : 64 * (hh % 2) + 64, p,
                                   512 * qb : 512 * qb + 512],
                            in0=usb[0:64, :],
                            in1=rec[:],
                            op=OP.mult,
                        )

            xt = load_x(0)
            vones = new_vones()
            qt = xp.tile([128, 3, T], dt.bfloat16, name="qt")
            kt = xp.tile([128, 3, T], dt.bfloat16, name="kt")
            for ti in range(NSI):
                v_unit(xt, vones, ti)()
            for tq in range(NTQ):
                for qk in range(2):
                    qk_unit(xt, qt, kt, 0, tq, qk)()

            prev = None  # (b, ot) with pending out-proj
            for b in range(BPC):
                ot = xp.tile([128, 3, T], dt.bfloat16, name="ot")
                for p in range(3):
                    if p < 2:
                        for tq in range(NTQ):
                            for qk in range(2):
                                fillers.append(qk_unit(xt, qt, kt, p + 1, tq, qk))
                    elif b + 1 < BPC:
                        nxt = load_x(b + 1)
                        nvones = new_vones()
                        nqt = xp.tile([128, 3, T], dt.bfloat16, name="qt")
                        nkt = xp.tile([128, 3, T], dt.bfloat16, name="kt")
                        for ti in range(NSI):
                            fillers.append(v_unit(nxt, nvones, ti))
                        for tq in range(NTQ):
                            for qk in range(2):
                                fillers.append(qk_unit(nxt, nqt, nkt, 0, tq, qk))
                    attention_pair(qt, kt, vones, ot, p)
                if prev is not None:
                    pb, pot = prev
                    for tq in range(NTQ):
                        for mo in range(KC):
                            fillers.append(oproj_unit(pb, pot, tq, mo))
                prev = (b, ot)
                if b + 1 < BPC:
                    xt, vones, qt, kt = nxt, nvones, nqt, nkt
            flush()
            pb, pot = prev
            for tq in range(NTQ):
                for mo in range(KC):
                    oproj_unit(pb, pot, tq, mo)()

    nc.compile()
    _CACHE["nc"] = nc
    return nc


def _prep_inputs(x, Wq, Wk, Wv, Wo, bo):
    import ml_dtypes
    bf16 = ml_dtypes.bfloat16
    x = np.ascontiguousarray(np.asarray(x, dtype=np.float32))
    Wq = np.asarray(Wq, dtype=np.float32)
    Wk = np.asarray(Wk, dtype=np.float32)
    Wv = np.asarray(Wv, dtype=np.float32)
    Wo = np.asarray(Wo, dtype=np.float32)
    bo = np.asarray(bo, dtype=np.float32)

    # x^T: [B, T, C] -> [B, C, T] -> [B, KC, 128, T]
    xT = np.ascontiguousarray(x.transpose(0, 2, 1)).reshape(B, KC, 128, T).astype(bf16)

    # Wqk columns per pair p: [Q_2p | Q_2p+1 | K_2p | K_2p+1], 64 each
    wqk = np.empty((C, 768), np.float32)
    for p in range(3):
        wqk[:, 256 * p + 0 : 256 * p + 64] = Wq[2 * p]
        wqk[:, 256 * p + 64 : 256 * p + 128] = Wq[2 * p + 1]
        wqk[:, 256 * p + 128 : 256 * p + 192] = Wk[2 * p]
        wqk[:, 256 * p + 192 : 256 * p + 256] = Wk[2 * p + 1]
    wqk = np.ascontiguousarray(wqk.reshape(KC, 128, 768)).astype(bf16)

    # Wv columns (h*64+e), rows C -> [KC, 128, 384]
    wv = np.ascontiguousarray(
        Wv.transpose(1, 0, 2).reshape(C, H * DH).reshape(KC, 128, H * DH)
    ).astype(bf16)
    wo = np.ascontiguousarray(Wo.reshape(KC, 128, C)).astype(bf16)
    bo_r = np.ascontiguousarray(bo.reshape(KC, 128, 1))
    return xT, wqk, wv, wo, bo_r


def _run(inputs, trace=False):
    from concourse.bass_utils import run_bass_kernel_spmd

    nc = _build()
    xT, wqk, wv, wo, bo_r = _prep_inputs(**inputs)
    in_maps = [
        {
            "xT": xT[BPC * i : BPC * (i + 1)],
            "Wqk": wqk,
            "Wv": wv,
            "Wo": wo,
            "bo": bo_r,
        }
        for i in range(NCORES)
    ]
    res = run_bass_kernel_spmd(nc, in_maps, list(range(NCORES)), trace=trace)
    # yT per core: [BPC, KC, 128, T] -> full y [B, T, C]
    yT = np.concatenate([np.asarray(res.results[i]["yT"]) for i in range(NCORES)], axis=0)
    y = yT.reshape(B, C, T).transpose(0, 2, 1)
    return np.ascontiguousarray(y.astype(np.float32)), res.exec_time_ns


def kernel(**inputs) -> np.ndarray:
    y, _ = _run(inputs, trace=False)
    return y
